# revision 9
# baseline (speedup 1.0000x reference)
"""Trainium2 Bass kernel for AttentionFusionModel (B=4, S=4096, D=200).

out = (attn(x1) + attn(x2)) @ Wo.T + bo, with attn sharing Wq/Wk/Wv.

Sharding: 8 (batch, modality) pairs -> 8 NeuronCores, one full self-attention
per core. Modality fusion = pairwise ReduceScatter between cores (2b, 2b+1),
each core projecting its own attention output first (projection is linear, so
proj(a1 + a2) = proj(a1) + proj(a2); softmax row-normalization commutes with
the projection and is applied post-projection as a per-row scale). The RS is
chunked so it overlaps the tail of the attention compute.

Per-core layout strategy (all big matmuls in bf16, fp32 PSUM accumulate):
  X^T [201, S]   (d on partitions, +ones row)  via DMA-transpose (bf16 xbar)
  Q^T, K^T [200, S] = (W·sc)^T-stationary matmuls (bias via ones-row aug)
  V [S, 201]     natural layout, +ones column (for sumexp)
  scores^T[k,q] tiles = K^T-slice.T @ Q^T   (contract d: 128+72 blocks)
  expT = Exp(scores^T)  on ScalarE (no max subtraction; |scores| ~< 7)
  U^T[d+1, q] += V'[k,:].T @ expT           (row 200 = sumexp L)
  proj[q, 201] = U^T-slice.T @ Wo'^T        (col 200 = L passthrough)
  out rows = proj[:, :200] * (1/L) + bo/2
"""

import os
import sys

sys.path.insert(0, "/opt/trn_rl_repo")

import numpy as np
from contextlib import ExitStack

from concourse import bacc, mybir, tile
from concourse.bass_utils import run_bass_kernel_spmd
from concourse.masks import make_identity

F32 = mybir.dt.float32
BF16 = mybir.dt.bfloat16
AF = mybir.ActivationFunctionType
ALU = mybir.AluOpType

B = 4
S = 4096
D = 200
P = 128
D2 = D - P            # 72
DA = D + 1            # 201 (augmented with ones row / sumexp col)
NCORES = 8
RG = [[0, 1], [2, 3], [4, 5], [6, 7]]  # core 2b+m handles (batch b, modality m)

USE_RS = True


def _emit_av(nc, acc1, acc2, v_sb, et, kb, nkb):
    k0 = kb * DA
    st = kb == 0
    sp = kb == nkb - 1
    nc.tensor.matmul(acc1[:], v_sb[:, k0:k0 + P], et[:], start=st, stop=sp)
    nc.tensor.matmul(acc2[:], v_sb[:, k0 + P:k0 + DA], et[:], start=st, stop=sp)


def _emit(ctx, tc, nc, exts, s_len, use_rs):
    x_ext, wq_ext, wk_ext, wv_ext, wo_ext, bo_ext, out_ext, rs_bufs = exts
    QG = min(512, s_len)
    nkb = s_len // P
    nqg = s_len // QG
    qg_per_chunk = min(2, nqg)
    nchunk = nqg // qg_per_chunk
    crows = qg_per_chunk * QG  # rs chunk input rows

    pers = ctx.enter_context(tc.tile_pool(name="pers", bufs=1))
    xt0 = pers.tile([P, s_len], BF16)
    xt1 = pers.tile([P, s_len], BF16)       # rows 0:72 = d 128:200, 72 = ones
    qt0 = pers.tile([P, s_len], BF16)
    qt1 = pers.tile([D2, s_len], BF16)
    kt0 = pers.tile([P, s_len], BF16)
    kt1 = pers.tile([D2, s_len], BF16)
    v_sb = pers.tile([P, nkb * DA], BF16)   # nkb tiles of [128, 201], col 200 = 1s
    wq0 = pers.tile([P, D], BF16)
    wq1 = pers.tile([D2 + 1, D], BF16)
    wk0 = pers.tile([P, D], BF16)
    wk1 = pers.tile([D2 + 1, D], BF16)
    wv0 = pers.tile([P, DA], BF16)
    wv1 = pers.tile([D2 + 1, DA], BF16)
    wo0 = pers.tile([P, DA], BF16)
    wo1 = pers.tile([D2 + 1, DA], BF16)
    bo_sb = pers.tile([P, D], F32)

    # ---- phase 1: load X, build X^T via PE transposes (bf16, 1 cyc/row) ----
    ident = pers.tile([P, P], BF16)
    make_identity(nc, ident[:])
    # ones row lives at partition 72 of xt1; engine APs need 32-aligned
    # partition bases, so memset [64:128) and let the X^T copies overwrite
    # rows 64..71 (rows 73.. stay harmless junk, never read).
    nc.vector.memset(xt1[64:P, :], 1.0)
    with ExitStack() as ph1:
        xp = ph1.enter_context(tc.tile_pool(name="xp", bufs=8))
        tps = ph1.enter_context(tc.tile_pool(name="tps", bufs=2, space="PSUM"))
        for n in range(nkb):
            c0, c1 = n * P, (n + 1) * P
            x_in = xp.tile([P, D], F32, tag="xin")
            nc.sync.dma_start(out=x_in[:], in_=x_ext[c0:c1, :])
            xc = xp.tile([P, D], BF16, tag="xc")
            nc.vector.tensor_copy(xc[:], x_in[:])
            p1 = tps.tile([P, P], BF16, tag="tp1")
            nc.tensor.transpose(p1[:], xc[:, 0:P], ident[:])
            p2 = tps.tile([D2, P], BF16, tag="tp2")
            nc.tensor.transpose(p2[:], xc[:, P:D], ident[:])
            nc.vector.tensor_copy(xt0[:, c0:c1], p1[:])
            nc.vector.tensor_copy(xt1[0:D2, c0:c1], p2[:])

        nc.sync.dma_start(out=bo_sb[:], in_=bo_ext[:, :])
        wst = ph1.enter_context(tc.tile_pool(name="wstage", bufs=2))
        for (ext, b0, b1, width) in [
            (wq_ext, wq0, wq1, D),
            (wk_ext, wk0, wk1, D),
            (wv_ext, wv0, wv1, DA),
            (wo_ext, wo0, wo1, DA),
        ]:
            wf0 = wst.tile([P, DA], F32, tag="wf0")
            wf1 = wst.tile([D2 + 1, DA], F32, tag="wf1")
            nc.sync.dma_start(out=wf0[:, 0:width], in_=ext[0:P, :])
            nc.sync.dma_start(out=wf1[:, 0:width], in_=ext[P:DA, :])
            nc.vector.tensor_copy(b0[:, 0:width], wf0[:, 0:width])
            nc.vector.tensor_copy(b1[:, 0:width], wf1[:, 0:width])

        # ---- QKV projections ----
        qkps = ph1.enter_context(tc.tile_pool(name="qkps", bufs=2, space="PSUM"))
        CH = min(512, s_len)
        for (w0, w1, t0, t1) in [(wq0, wq1, qt0, qt1), (wk0, wk1, kt0, kt1)]:
            for ob, obw in [(0, P), (1, D2)]:
                tdst = t0 if ob == 0 else t1
                for ch in range(s_len // CH):
                    c0, c1 = ch * CH, (ch + 1) * CH
                    ps = qkps.tile([P, CH], F32, tag="qk")
                    nc.tensor.matmul(ps[0:obw, :], w0[:, ob * P:ob * P + obw],
                                     xt0[:, c0:c1], start=True, stop=False)
                    nc.tensor.matmul(ps[0:obw, :], w1[:, ob * P:ob * P + obw],
                                     xt1[0:D2 + 1, c0:c1], start=False, stop=True)
                    nc.vector.tensor_copy(tdst[:, c0:c1], ps[0:obw, :])

        vps = ph1.enter_context(tc.tile_pool(name="vps", bufs=2, space="PSUM"))
        for n in range(nkb):
            c0, c1 = n * P, (n + 1) * P
            pv = vps.tile([P, DA], F32, tag="pv")
            nc.tensor.matmul(pv[:], xt0[:, c0:c1], wv0[:], start=True, stop=False)
            nc.tensor.matmul(pv[:], xt1[0:D2 + 1, c0:c1], wv1[:],
                             start=False, stop=True)
            nc.vector.tensor_copy(v_sb[:, n * DA:(n + 1) * DA], pv[:])

    # ---- phase 2: attention + projection + epilogue (+ chunked RS) ----
    with ExitStack() as ph2:
        scp = ph2.enter_context(tc.tile_pool(name="scp", bufs=2, space="PSUM"))
        accp = ph2.enter_context(tc.tile_pool(name="accp", bufs=2, space="PSUM"))
        projp = ph2.enter_context(tc.tile_pool(name="projp", bufs=2, space="PSUM"))
        etp = ph2.enter_context(tc.tile_pool(name="etp", bufs=3))
        utp = ph2.enter_context(tc.tile_pool(name="utp", bufs=2))
        epip = ph2.enter_context(tc.tile_pool(name="epip", bufs=4))
        for qg in range(nqg):
            q0, q1 = qg * QG, (qg + 1) * QG
            acc1 = accp.tile([P, QG], F32, tag="acc1")
            acc2 = accp.tile([D2 + 1, QG], F32, tag="acc2")
            ets = {}
            for kb in range(nkb):
                k0 = kb * P
                sc_ps = scp.tile([P, QG], F32, tag="sc")
                nc.tensor.matmul(sc_ps[:], kt0[:, k0:k0 + P], qt0[:, q0:q1],
                                 start=True, stop=False)
                nc.tensor.matmul(sc_ps[:], kt1[:, k0:k0 + P], qt1[:, q0:q1],
                                 start=False, stop=True)
                et = etp.tile([P, QG], BF16, tag="et")
                nc.scalar.activation(et[:], sc_ps[:], AF.Exp)
                ets[kb] = et
                if kb >= 1:
                    _emit_av(nc, acc1, acc2, v_sb, ets.pop(kb - 1), kb - 1, nkb)
            _emit_av(nc, acc1, acc2, v_sb, ets.pop(nkb - 1), nkb - 1, nkb)

            ut0 = utp.tile([P, QG], BF16, tag="ut0")
            ut1 = utp.tile([D2 + 1, QG], BF16, tag="ut1")
            nc.vector.tensor_copy(ut0[:], acc1[:])
            nc.vector.tensor_copy(ut1[:], acc2[:])
            chunk = qg // qg_per_chunk
            for qb in range(QG // P):
                pp = projp.tile([P, DA], F32, tag="pp")
                nc.tensor.matmul(pp[:], ut0[:, qb * P:(qb + 1) * P], wo0[:],
                                 start=True, stop=False)
                nc.tensor.matmul(pp[:], ut1[:, qb * P:(qb + 1) * P], wo1[:],
                                 start=False, stop=True)
                rc = epip.tile([P, 1], F32, tag="rc")
                nc.vector.reciprocal(rc[:], pp[:, D:DA])
                ot = epip.tile([P, D], F32, tag="ot")
                nc.vector.tensor_scalar(ot[:], pp[:, 0:D], rc[:], None, ALU.mult)
                nc.vector.tensor_tensor(ot[:], ot[:], bo_sb[:], ALU.add)
                r0 = q0 + qb * P
                if use_rs:
                    dst = rs_bufs[chunk][0]
                    nc.sync.dma_start(
                        out=dst[r0 - chunk * crows:r0 - chunk * crows + P, :],
                        in_=ot[:])
                else:
                    nc.sync.dma_start(out=out_ext[r0:r0 + P, :], in_=ot[:])

            if use_rs and (qg + 1) % qg_per_chunk == 0:
                ci, co = rs_bufs[chunk]
                nc.gpsimd.collective_compute(
                    "ReduceScatter",
                    ALU.add,
                    replica_groups=RG,
                    ins=[ci[:, :].opt()],
                    outs=[co[:, :].opt()],
                )
                orow = chunk * (crows // 2)
                nc.sync.dma_start(out=out_ext[orow:orow + crows // 2, :],
                                  in_=co[:, :])


_CACHE = {}


def _build(s_len=S, use_rs=USE_RS):
    key = (s_len, use_rs)
    if key not in _CACHE:
        nc = bacc.Bacc("TRN2", target_bir_lowering=False, debug=False,
                       num_devices=NCORES)
        x_ext = nc.dram_tensor("x", [s_len, D], F32, kind="ExternalInput")
        wq_ext = nc.dram_tensor("wq", [DA, D], F32, kind="ExternalInput")
        wk_ext = nc.dram_tensor("wk", [DA, D], F32, kind="ExternalInput")
        wv_ext = nc.dram_tensor("wv", [DA, DA], F32, kind="ExternalInput")
        wo_ext = nc.dram_tensor("wo", [DA, DA], F32, kind="ExternalInput")
        bo_ext = nc.dram_tensor("bo", [P, D], F32, kind="ExternalInput")
        out_rows = s_len // 2 if use_rs else s_len
        out_ext = nc.dram_tensor("out", [out_rows, D], F32, kind="ExternalOutput")
        rs_bufs = []
        if use_rs:
            QG = min(512, s_len)
            nqg = s_len // QG
            qg_per_chunk = min(2, nqg)
            nchunk = nqg // qg_per_chunk
            crows = qg_per_chunk * QG
            for g in range(nchunk):
                ci = nc.dram_tensor(f"rs_in{g}", [crows, D], F32)
                co = nc.dram_tensor(f"rs_out{g}", [crows // 2, D], F32)
                rs_bufs.append((ci, co))
        exts = (x_ext, wq_ext, wk_ext, wv_ext, wo_ext, bo_ext, out_ext, rs_bufs)
        with tile.TileContext(nc) as tc:
            with ExitStack() as ctx:
                _emit(ctx, tc, nc, exts, s_len, use_rs)
        nc.compile()
        _CACHE[key] = nc
    return _CACHE[key]


def _prep_in_maps(m1, m2, Wq, bq, Wk, bk, Wv, bv, Wo, bo, s_len=S):
    sc = np.float32(1.0 / np.sqrt(D))
    wq_p = np.zeros((DA, D), np.float32)
    wq_p[:D] = Wq.T * sc
    wq_p[D] = bq * sc
    wk_p = np.zeros((DA, D), np.float32)
    wk_p[:D] = Wk.T
    wk_p[D] = bk
    wv_p = np.zeros((DA, DA), np.float32)
    wv_p[:D, :D] = Wv.T
    wv_p[D, :D] = bv
    wv_p[D, D] = 1.0
    wo_p = np.zeros((DA, DA), np.float32)
    wo_p[:D, :D] = Wo.T
    wo_p[D, D] = 1.0
    bo_t = np.ascontiguousarray(
        np.broadcast_to((bo * 0.5).astype(np.float32), (P, D)))
    in_maps = []
    for c in range(NCORES):
        b, m = c // 2, c % 2
        x = (m1 if m == 0 else m2)[b][:s_len]
        in_maps.append({
            "x": np.ascontiguousarray(x, np.float32),
            "wq": wq_p, "wk": wk_p, "wv": wv_p, "wo": wo_p, "bo": bo_t,
        })
    return in_maps


def _run(inputs, s_len=S, use_rs=USE_RS, trace=False, tmpdir=None):
    m1 = np.asarray(inputs["modal1_input"], np.float32)
    m2 = np.asarray(inputs["modal2_input"], np.float32)
    args = [np.asarray(inputs[k], np.float32)
            for k in ("Wq", "bq", "Wk", "bk", "Wv", "bv", "Wo", "bo")]
    nc = _build(s_len, use_rs)
    in_maps = _prep_in_maps(m1, m2, *args, s_len=s_len)
    kr = run_bass_kernel_spmd(nc, in_maps, core_ids=list(range(NCORES)),
                              trace=trace, tmpdir=tmpdir)
    res = kr.results
    out = np.empty((B, s_len, D), np.float32)
    if use_rs:
        # chunked RS: core 2b holds the first half of every chunk, core 2b+1
        # the second half; chunk g covers global rows [g*crows, (g+1)*crows)
        QG = min(512, s_len)
        nqg = s_len // QG
        crows = min(2, nqg) * QG
        csz = crows // 2
        nchunk = s_len // crows
        for b in range(B):
            for g in range(nchunk):
                lo, hi = g * csz, (g + 1) * csz
                out[b, g * crows:g * crows + csz] = res[2 * b]["out"][lo:hi]
                out[b, g * crows + csz:(g + 1) * crows] = \
                    res[2 * b + 1]["out"][lo:hi]
    else:
        for b in range(B):
            out[b] = res[2 * b]["out"] + res[2 * b + 1]["out"]
    return out, kr


def kernel(**inputs):
    out, _ = _run(inputs)
    return out



# revision 18
# speedup vs baseline: 1.1552x; 1.1552x over previous
"""Trainium2 Bass kernel for AttentionFusionModel (B=4, S=4096, D=200).

out = (attn(x1) + attn(x2)) @ Wo.T + bo, with attn sharing Wq/Wk/Wv.

Sharding: 8 (batch, modality) pairs -> 8 NeuronCores, one full self-attention
per core. Modality fusion = pairwise ReduceScatter between cores (2b, 2b+1)
in 512-row chunks (bf16) overlapping the compute tail. Each core projects its
own attention output first (projection is linear; softmax row-normalization
commutes and is applied post-projection as a per-row scale).

Key tricks:
  - scores = x̂ A x̂^T with A = Ŵq Ŵk^T folded on the HOST: the K projection
    disappears; z = x̂ A plays the role of Q (same cost/precision as before).
  - X^T [201, S] pre-transposed + bf16-cast + ones-row-augmented on HOST,
    DMA'd in [32-partition x 1024-column] chunks across the 8 DMA queues
    (weights queued FIRST so the first matmul isn't stuck behind 1.6MB).
  - Paired k-blocks: one [128, 1024] f32 PSUM tile (2 banks) per two
    k-blocks, exp'd in two half ACTIVATEs. PSUM: sc 2x2 + acc 2 + proj 2 = 8.
  - V [S, 201] natural with ones column -> row 200 of U^T = sumexp L.
  - proj for q-group g is deferred into group g+1's loop so the PE never
    waits on the accumulator PSUM->SBUF copies.
  - warm-up matmuls during the input DMA keep the PE HAM clock at 2.4 GHz.
"""

import sys

sys.path.insert(0, "/opt/trn_rl_repo")

import numpy as np
import ml_dtypes
from contextlib import ExitStack

from concourse import bacc, mybir, tile
from concourse.bass_utils import run_bass_kernel_spmd

F32 = mybir.dt.float32
BF16 = mybir.dt.bfloat16
AF = mybir.ActivationFunctionType
ALU = mybir.AluOpType

B = 4
S = 4096
D = 200
P = 128
D2 = D - P            # 72
DA = D + 1            # 201 (augmented with ones row / sumexp col)
DB = DA - P           # 73
NCORES = 8
RG = [[0, 1], [2, 3], [4, 5], [6, 7]]  # core 2b+m handles (batch b, modality m)
QG = 512              # q columns per attention group (1 PSUM bank fp32)
CROWS = 512           # rows per ReduceScatter chunk
RS_DT = BF16          # dtype for the fusion ReduceScatter


def _emit(ctx, tc, nc, exts, s_len):
    (xt0_ext, xt1_ext, wz0_ext, wz1_ext, wv0_ext, wv1_ext,
     wo0_ext, wo1_ext, bo_ext, out_ext, rs_bufs) = exts
    nkb = s_len // P              # 32 k-blocks
    npr = nkb // 2                # 16 k-block pairs
    nqg = s_len // QG             # 8 q-groups
    qb_per_qg = QG // P           # 4

    pers = ctx.enter_context(tc.tile_pool(name="pers", bufs=1))
    xt0 = pers.tile([P, s_len], BF16)
    xt1 = pers.tile([DB, s_len], BF16)   # rows 0:72 = d 128:200, row 72 = ones
    zt0 = pers.tile([P, s_len], BF16)    # z = x̂ A  (plays the role of Q)
    zt1 = pers.tile([DB, s_len], BF16)
    v_sb = pers.tile([P, nkb * DA], BF16)  # nkb tiles [128, 201], col 200 = 1s
    wz0 = pers.tile([P, DA], BF16)
    wz1 = pers.tile([DB, DA], BF16)
    wv0 = pers.tile([P, DA], BF16)
    wv1 = pers.tile([DB, DA], BF16)
    wo0 = pers.tile([P, DA], BF16)
    wo1 = pers.tile([DB, DA], BF16)
    bo_sb = pers.tile([P, D], F32)
    junk = pers.tile([P, QG], BF16)

    # ---- phase 1: weights DMA first, then X^T in 2D chunks ----
    for dst, ext in [(wz0, wz0_ext), (wz1, wz1_ext), (wv0, wv0_ext),
                     (wv1, wv1_ext), (wo0, wo0_ext), (wo1, wo1_ext)]:
        nc.sync.dma_start(out=dst[:], in_=ext[:, :])
    nc.sync.dma_start(out=bo_sb[:], in_=bo_ext[:, :])
    nc.vector.memset(junk[:], 0.5)

    DCH = min(1024, s_len)
    for c in range(s_len // DCH):
        c0, c1 = c * DCH, (c + 1) * DCH
        for p0, p1 in [(0, 32), (32, 64), (64, 96), (96, 128)]:
            nc.sync.dma_start(out=xt0[p0:p1, c0:c1], in_=xt0_ext[p0:p1, c0:c1])
        for p0, p1 in [(0, 32), (32, 64), (64, DB)]:
            nc.sync.dma_start(out=xt1[p0:p1, c0:c1], in_=xt1_ext[p0:p1, c0:c1])

    with ExitStack() as ph1:
        wup = ph1.enter_context(tc.tile_pool(name="wup", bufs=1, space="PSUM"))
        wps = wup.tile([P, QG], F32)
        for _ in range(50):     # ~3.9us of junk matmuls: HAM warm-up while
            nc.tensor.matmul(wps[:], junk[:, 0:P], junk[:],  # the X DMA lands
                             start=True, stop=True, skip_group_check=True)

        qkps = ph1.enter_context(tc.tile_pool(name="qkps", bufs=2, space="PSUM"))
        for ch in range(s_len // DCH):
            c0 = ch * DCH
            for ob, obw in [(0, P), (1, DB)]:
                tdst = zt0 if ob == 0 else zt1
                ps = qkps.tile([P, DCH], F32, tag="qk")
                for h in range(DCH // QG):   # matmul out <= 1 PSUM bank
                    h0 = h * QG
                    nc.tensor.matmul(ps[0:obw, h0:h0 + QG],
                                     wz0[:, ob * P:ob * P + obw],
                                     xt0[:, c0 + h0:c0 + h0 + QG],
                                     start=True, stop=False)
                    nc.tensor.matmul(ps[0:obw, h0:h0 + QG],
                                     wz1[:, ob * P:ob * P + obw],
                                     xt1[:, c0 + h0:c0 + h0 + QG],
                                     start=False, stop=True)
                if ob == 0:
                    nc.vector.tensor_copy(tdst[:, c0:c0 + DCH], ps[0:obw, :])
                else:
                    nc.scalar.copy(tdst[:, c0:c0 + DCH], ps[0:obw, :])

    # ---- phase 2: attention + deferred projection + chunked RS ----
    with ExitStack() as ph2:
        scp = ph2.enter_context(tc.tile_pool(name="scp", bufs=2, space="PSUM"))
        accp = ph2.enter_context(tc.tile_pool(name="accp", bufs=1, space="PSUM"))
        projp = ph2.enter_context(tc.tile_pool(name="projp", bufs=2, space="PSUM"))
        etp = ph2.enter_context(tc.tile_pool(name="etp", bufs=3))
        utp = ph2.enter_context(tc.tile_pool(name="utp", bufs=2))
        epip = ph2.enter_context(tc.tile_pool(name="epip", bufs=4))

        def emit_v(n):
            # V-projection block n, interleaved into qg0's loop (projp pool is
            # free until qg1 thanks to the proj deferral)
            c0, c1 = n * P, (n + 1) * P
            pv = projp.tile([P, DA], F32, tag="pp")
            nc.tensor.matmul(pv[:], xt0[:, c0:c1], wv0[:], start=True, stop=False)
            nc.tensor.matmul(pv[:], xt1[:, c0:c1], wv1[:], start=False, stop=True)
            if n % 2 == 0:
                nc.vector.tensor_copy(v_sb[:, n * DA:(n + 1) * DA], pv[:])
            else:
                nc.scalar.copy(v_sb[:, n * DA:(n + 1) * DA], pv[:])

        def emit_av(acc1, acc2, et, pr):
            for j in range(2):
                kb = 2 * pr + j
                k0 = kb * DA
                st = kb == 0
                sp = kb == nkb - 1
                ets = et[:, j * QG:(j + 1) * QG]
                nc.tensor.matmul(acc1[:], v_sb[:, k0:k0 + P], ets,
                                 start=st, stop=sp)
                nc.tensor.matmul(acc2[:], v_sb[:, k0 + P:k0 + DA], ets,
                                 start=st, stop=sp)

        def emit_proj(qg, ut0, ut1):
            q0 = qg * QG
            for qb in range(qb_per_qg):
                pp = projp.tile([P, DA], F32, tag="pp")
                nc.tensor.matmul(pp[:], ut0[:, qb * P:(qb + 1) * P], wo0[:],
                                 start=True, stop=False)
                nc.tensor.matmul(pp[:], ut1[:, qb * P:(qb + 1) * P], wo1[:],
                                 start=False, stop=True)
                rc = epip.tile([P, 1], F32, tag="rc")
                nc.vector.reciprocal(rc[:], pp[:, D:DA])
                ot = epip.tile([P, D], RS_DT, tag="ot")
                nc.vector.tensor_scalar(ot[:], pp[:, 0:D], rc[:], None, ALU.mult)
                nc.vector.tensor_tensor(ot[:], ot[:], bo_sb[:], ALU.add)
                r0 = q0 + qb * P
                chunk = r0 // CROWS
                roff = r0 - chunk * CROWS
                nc.sync.dma_start(out=rs_bufs[chunk][0][roff:roff + P, :],
                                  in_=ot[:])
                if roff + P == CROWS:
                    ci, co = rs_bufs[chunk]
                    orow = chunk * (CROWS // 2)
                    nc.gpsimd.collective_compute(
                        "ReduceScatter",
                        ALU.add,
                        replica_groups=RG,
                        ins=[ci[:, :].opt()],
                        outs=[co[:, :].opt()],
                    )
                    nc.sync.dma_start(
                        out=out_ext[orow:orow + CROWS // 2, :], in_=co[:, :])

        pending = None
        for qg in range(nqg):
            q0, q1 = qg * QG, (qg + 1) * QG
            acc1 = accp.tile([P, QG], F32, tag="acc1")
            acc2 = accp.tile([DB, QG], F32, tag="acc2")
            ets = {}
            for pr in range(npr):
                sc = scp.tile([P, 2 * QG], F32, tag="sc")
                for j in range(2):
                    kb = 2 * pr + j
                    k0 = kb * P
                    scj = sc[:, j * QG:(j + 1) * QG]
                    nc.tensor.matmul(scj, xt0[:, k0:k0 + P], zt0[:, q0:q1],
                                     start=True, stop=False)
                    nc.tensor.matmul(scj, xt1[:, k0:k0 + P], zt1[:, q0:q1],
                                     start=False, stop=True)
                et = etp.tile([P, 2 * QG], BF16, tag="et")
                nc.scalar.activation(et[:, 0:QG], sc[:, 0:QG], AF.Exp)
                nc.scalar.activation(et[:, QG:2 * QG], sc[:, QG:2 * QG], AF.Exp)
                ets[pr] = et
                if qg == 0:
                    emit_v(2 * pr)
                    emit_v(2 * pr + 1)
                if pr == 2 and pending is not None:
                    emit_proj(*pending)
                    pending = None
                if pr >= 1:
                    emit_av(acc1, acc2, ets.pop(pr - 1), pr - 1)
            emit_av(acc1, acc2, ets.pop(npr - 1), npr - 1)

            ut0 = utp.tile([P, QG], BF16, tag="ut0")
            ut1 = utp.tile([DB, QG], BF16, tag="ut1")
            nc.vector.tensor_copy(ut0[:], acc1[:])
            nc.vector.tensor_copy(ut1[:], acc2[:])
            pending = (qg, ut0, ut1)
        emit_proj(*pending)


_CACHE = {}


def _build(s_len=S):
    if s_len not in _CACHE:
        nc = bacc.Bacc("TRN2", target_bir_lowering=False, debug=False,
                       num_devices=NCORES)
        xt0_ext = nc.dram_tensor("xt0", [P, s_len], BF16, kind="ExternalInput")
        xt1_ext = nc.dram_tensor("xt1", [DB, s_len], BF16, kind="ExternalInput")
        w_exts = []
        for nm, rows in [("wz0", P), ("wz1", DB), ("wv0", P), ("wv1", DB),
                         ("wo0", P), ("wo1", DB)]:
            w_exts.append(nc.dram_tensor(nm, [rows, DA], BF16,
                                         kind="ExternalInput"))
        bo_ext = nc.dram_tensor("bo", [P, D], F32, kind="ExternalInput")
        out_ext = nc.dram_tensor("out", [s_len // 2, D], RS_DT,
                                 kind="ExternalOutput")
        rs_bufs = []
        for g in range(s_len // CROWS):
            ci = nc.dram_tensor(f"rs_in{g}", [CROWS, D], RS_DT)
            co = nc.dram_tensor(f"rs_out{g}", [CROWS // 2, D], RS_DT)
            rs_bufs.append((ci, co))
        exts = (xt0_ext, xt1_ext, *w_exts, bo_ext, out_ext, rs_bufs)
        with tile.TileContext(nc) as tc:
            with ExitStack() as ctx:
                _emit(ctx, tc, nc, exts, s_len)
        nc.compile()
        _CACHE[s_len] = nc
    return _CACHE[s_len]


def _prep_in_maps(m1, m2, Wq, bq, Wk, bk, Wv, bv, Wo, bo, s_len=S):
    bf = ml_dtypes.bfloat16
    sc = np.float64(1.0 / np.sqrt(D))
    wq_a = np.concatenate([Wq.T.astype(np.float64) * sc,
                           (bq.astype(np.float64) * sc)[None, :]], 0)
    wk_a = np.concatenate([Wk.T.astype(np.float64),
                           bk.astype(np.float64)[None, :]], 0)
    A = (wq_a @ wk_a.T).astype(bf)          # [201, 201]: scores = x̂ A x̂^T
    wv_p = np.zeros((DA, DA), np.float32)
    wv_p[:D, :D] = Wv.T
    wv_p[D, :D] = bv
    wv_p[D, D] = 1.0
    wv_p = wv_p.astype(bf)
    wo_p = np.zeros((DA, DA), np.float32)
    wo_p[:D, :D] = Wo.T
    wo_p[D, D] = 1.0
    wo_p = wo_p.astype(bf)
    bo_t = np.ascontiguousarray(
        np.broadcast_to((bo * 0.5).astype(np.float32), (P, D)))
    weights = {
        "wz0": np.ascontiguousarray(A[:P]),
        "wz1": np.ascontiguousarray(A[P:]),
        "wv0": np.ascontiguousarray(wv_p[:P]),
        "wv1": np.ascontiguousarray(wv_p[P:]),
        "wo0": np.ascontiguousarray(wo_p[:P]),
        "wo1": np.ascontiguousarray(wo_p[P:]),
        "bo": bo_t,
    }
    ones = np.ones((1, s_len), np.float32)
    in_maps = []
    for c in range(NCORES):
        b, m = c // 2, c % 2
        x = (m1 if m == 0 else m2)[b][:s_len]             # [s, 200] f32
        xt = np.concatenate([x.T, ones], 0).astype(bf)    # [201, s] bf16
        in_maps.append({
            "xt0": np.ascontiguousarray(xt[:P]),
            "xt1": np.ascontiguousarray(xt[P:]),
            **weights,
        })
    return in_maps


def _run(inputs, s_len=S, trace=False, tmpdir=None):
    m1 = np.asarray(inputs["modal1_input"], np.float32)
    m2 = np.asarray(inputs["modal2_input"], np.float32)
    args = [np.asarray(inputs[k], np.float32)
            for k in ("Wq", "bq", "Wk", "bk", "Wv", "bv", "Wo", "bo")]
    nc = _build(s_len)
    in_maps = _prep_in_maps(m1, m2, *args, s_len=s_len)
    kr = run_bass_kernel_spmd(nc, in_maps, core_ids=list(range(NCORES)),
                              trace=trace, tmpdir=tmpdir)
    res = kr.results
    out = np.empty((B, s_len, D), np.float32)
    csz = CROWS // 2
    nchunk = s_len // CROWS
    for b in range(B):
        e = np.asarray(res[2 * b]["out"], np.float32)
        o = np.asarray(res[2 * b + 1]["out"], np.float32)
        for g in range(nchunk):
            out[b, g * CROWS:g * CROWS + csz] = e[g * csz:(g + 1) * csz]
            out[b, g * CROWS + csz:(g + 1) * CROWS] = o[g * csz:(g + 1) * csz]
    return out, kr


def kernel(**inputs):
    out, _ = _run(inputs)
    return out


# revision 19
# speedup vs baseline: 1.3547x; 1.1726x over previous
"""Trainium2 Bass kernel for AttentionFusionModel (B=4, S=4096, D=200).

out = (attn(x1) + attn(x2)) @ Wo.T + bo, with attn sharing Wq/Wk/Wv.

Sharding: 8 (batch, modality) pairs -> 8 NeuronCores, one full self-attention
per core. Modality fusion = pairwise ReduceScatter between cores (2b, 2b+1)
in 512-row chunks (bf16) overlapping the compute tail. Each core projects its
own attention output first (projection is linear; softmax row-normalization
commutes and is applied post-projection as a per-row scale).

Key tricks:
  - scores = x̂ A x̂^T with A = Ŵq Ŵk^T folded on the HOST: the K projection
    disappears; z = x̂ A plays the role of Q (same cost/precision as before).
  - X^T [201, S] pre-transposed + bf16-cast + ones-row-augmented on HOST,
    DMA'd in [32-partition x 1024-column] chunks across the 8 DMA queues
    (weights queued FIRST so the first matmul isn't stuck behind 1.6MB).
  - Paired k-blocks: one [128, 1024] f32 PSUM tile (2 banks) per two
    k-blocks, exp'd in two half ACTIVATEs. PSUM: sc 2x2 + acc 2 + proj 2 = 8.
  - V [S, 201] natural with ones column -> row 200 of U^T = sumexp L.
  - proj for q-group g is deferred into group g+1's loop so the PE never
    waits on the accumulator PSUM->SBUF copies.
  - warm-up matmuls during the input DMA keep the PE HAM clock at 2.4 GHz.
"""

import sys

sys.path.insert(0, "/opt/trn_rl_repo")

import numpy as np
import ml_dtypes
from contextlib import ExitStack

from concourse import bacc, mybir, tile
from concourse.bass_utils import run_bass_kernel_spmd

F32 = mybir.dt.float32
BF16 = mybir.dt.bfloat16
AF = mybir.ActivationFunctionType
ALU = mybir.AluOpType

B = 4
S = 4096
D = 200
P = 128
D2 = D - P            # 72
DA = D + 1            # 201 (augmented with ones row / sumexp col)
DB = DA - P           # 73
NCORES = 8
RG = [[0, 1], [2, 3], [4, 5], [6, 7]]  # core 2b+m handles (batch b, modality m)
QG = 512              # q columns per attention group (1 PSUM bank fp32)
CROWS = 512           # rows per ReduceScatter chunk
RS_DT = BF16          # dtype for the fusion ReduceScatter


def _emit(ctx, tc, nc, exts, s_len):
    (xt0_ext, xt1_ext, wz0_ext, wz1_ext, wv0_ext, wv1_ext,
     wo0_ext, wo1_ext, bo_ext, out_ext, rs_bufs) = exts
    nkb = s_len // P              # 32 k-blocks
    npr = nkb // 2                # 16 k-block pairs
    nqg = s_len // QG             # 8 q-groups
    qb_per_qg = QG // P           # 4

    pers = ctx.enter_context(tc.tile_pool(name="pers", bufs=1))
    xt0 = pers.tile([P, s_len], BF16)
    xt1 = pers.tile([DB, s_len], BF16)   # rows 0:72 = d 128:200, row 72 = ones
    zt0 = pers.tile([P, s_len], BF16)    # z = x̂ A  (plays the role of Q)
    zt1 = pers.tile([DB, s_len], BF16)
    v_sb = pers.tile([P, nkb * DA], BF16)  # nkb tiles [128, 201], col 200 = 1s
    wz0 = pers.tile([P, DA], BF16)
    wz1 = pers.tile([DB, DA], BF16)
    wv0 = pers.tile([P, DA], BF16)
    wv1 = pers.tile([DB, DA], BF16)
    wo0 = pers.tile([P, DA], BF16)
    wo1 = pers.tile([DB, DA], BF16)
    bo_sb = pers.tile([P, D], F32)
    junk = pers.tile([P, QG], BF16)

    # ---- phase 1: weights DMA first, then X^T in 2D chunks ----
    for dst, ext in [(wz0, wz0_ext), (wz1, wz1_ext), (wv0, wv0_ext),
                     (wv1, wv1_ext), (wo0, wo0_ext), (wo1, wo1_ext)]:
        nc.sync.dma_start(out=dst[:], in_=ext[:, :])
    nc.sync.dma_start(out=bo_sb[:], in_=bo_ext[:, :])
    nc.vector.memset(junk[:], 0.5)

    DCH = min(1024, s_len)
    for c in range(s_len // DCH):
        c0, c1 = c * DCH, (c + 1) * DCH
        for p0, p1 in [(0, 32), (32, 64), (64, 96), (96, 128)]:
            nc.sync.dma_start(out=xt0[p0:p1, c0:c1], in_=xt0_ext[p0:p1, c0:c1])
        for p0, p1 in [(0, 32), (32, 64), (64, DB)]:
            nc.sync.dma_start(out=xt1[p0:p1, c0:c1], in_=xt1_ext[p0:p1, c0:c1])

    with ExitStack() as ph1:
        wup = ph1.enter_context(tc.tile_pool(name="wup", bufs=1, space="PSUM"))
        wps = wup.tile([P, QG], F32)
        for _ in range(9):     # ~3.9us of junk matmuls: HAM warm-up while
            nc.tensor.matmul(wps[:], junk[:, 0:P], junk[:],  # the X DMA lands
                             start=True, stop=True, skip_group_check=True)

        qkps = ph1.enter_context(tc.tile_pool(name="qkps", bufs=2, space="PSUM"))
        for ch in range(s_len // DCH):
            c0 = ch * DCH
            for ob, obw in [(0, P), (1, DB)]:
                tdst = zt0 if ob == 0 else zt1
                ps = qkps.tile([P, DCH], F32, tag="qk")
                for h in range(DCH // QG):   # matmul out <= 1 PSUM bank
                    h0 = h * QG
                    nc.tensor.matmul(ps[0:obw, h0:h0 + QG],
                                     wz0[:, ob * P:ob * P + obw],
                                     xt0[:, c0 + h0:c0 + h0 + QG],
                                     start=True, stop=False)
                    nc.tensor.matmul(ps[0:obw, h0:h0 + QG],
                                     wz1[:, ob * P:ob * P + obw],
                                     xt1[:, c0 + h0:c0 + h0 + QG],
                                     start=False, stop=True)
                if ob == 0:
                    nc.vector.tensor_copy(tdst[:, c0:c0 + DCH], ps[0:obw, :])
                else:
                    nc.scalar.copy(tdst[:, c0:c0 + DCH], ps[0:obw, :])

    # ---- phase 2: attention + deferred projection + chunked RS ----
    with ExitStack() as ph2:
        scp = ph2.enter_context(tc.tile_pool(name="scp", bufs=2, space="PSUM"))
        accp = ph2.enter_context(tc.tile_pool(name="accp", bufs=1, space="PSUM"))
        projp = ph2.enter_context(tc.tile_pool(name="projp", bufs=2, space="PSUM"))
        etp = ph2.enter_context(tc.tile_pool(name="etp", bufs=3))
        utp = ph2.enter_context(tc.tile_pool(name="utp", bufs=2))
        epip = ph2.enter_context(tc.tile_pool(name="epip", bufs=4))

        def emit_v(n):
            # V-projection block n, interleaved into qg0's loop (projp pool is
            # free until qg1 thanks to the proj deferral)
            c0, c1 = n * P, (n + 1) * P
            pv = projp.tile([P, DA], F32, tag="pp")
            nc.tensor.matmul(pv[:], xt0[:, c0:c1], wv0[:], start=True, stop=False)
            nc.tensor.matmul(pv[:], xt1[:, c0:c1], wv1[:], start=False, stop=True)
            if n % 2 == 0:
                nc.vector.tensor_copy(v_sb[:, n * DA:(n + 1) * DA], pv[:])
            else:
                nc.scalar.copy(v_sb[:, n * DA:(n + 1) * DA], pv[:])

        def emit_av(acc1, acc2, et, pr):
            for j in range(2):
                kb = 2 * pr + j
                k0 = kb * DA
                st = kb == 0
                sp = kb == nkb - 1
                ets = et[:, j * QG:(j + 1) * QG]
                nc.tensor.matmul(acc1[:], v_sb[:, k0:k0 + P], ets,
                                 start=st, stop=sp)
                nc.tensor.matmul(acc2[:], v_sb[:, k0 + P:k0 + DA], ets,
                                 start=st, stop=sp)

        def emit_proj(qg, ut0, ut1):
            q0 = qg * QG
            last = qg == nqg - 1
            for qb in range(qb_per_qg):
                pp = projp.tile([P, DA], F32, tag="pp")
                nc.tensor.matmul(pp[:], ut0[:, qb * P:(qb + 1) * P], wo0[:],
                                 start=True, stop=False)
                nc.tensor.matmul(pp[:], ut1[:, qb * P:(qb + 1) * P], wo1[:],
                                 start=False, stop=True)
                rc = epip.tile([P, 1], F32, tag="rc")
                nc.vector.reciprocal(rc[:], pp[:, D:DA])
                ot = epip.tile([P, D], RS_DT, tag="ot")
                if last:
                    # ACT does the scale (ScE is faster from PSUM and the DVE
                    # chain would otherwise gate the final ReduceScatter)
                    nc.scalar.activation(ot[:], pp[:, 0:D], AF.Copy,
                                         scale=rc[:])
                else:
                    nc.vector.tensor_scalar(ot[:], pp[:, 0:D], rc[:], None,
                                            ALU.mult)
                nc.vector.tensor_tensor(ot[:], ot[:], bo_sb[:], ALU.add)
                r0 = q0 + qb * P
                chunk = r0 // CROWS
                roff = r0 - chunk * CROWS
                nc.sync.dma_start(out=rs_bufs[chunk][0][roff:roff + P, :],
                                  in_=ot[:])
                if roff + P == CROWS:
                    ci, co = rs_bufs[chunk]
                    orow = chunk * (CROWS // 2)
                    nc.gpsimd.collective_compute(
                        "ReduceScatter",
                        ALU.add,
                        replica_groups=RG,
                        ins=[ci[:, :].opt()],
                        outs=[co[:, :].opt()],
                    )
                    nc.sync.dma_start(
                        out=out_ext[orow:orow + CROWS // 2, :], in_=co[:, :])

        pending = None
        for qg in range(nqg):
            q0, q1 = qg * QG, (qg + 1) * QG
            acc1 = accp.tile([P, QG], F32, tag="acc1")
            acc2 = accp.tile([DB, QG], F32, tag="acc2")
            ets = {}
            for pr in range(npr):
                sc = scp.tile([P, 2 * QG], F32, tag="sc")
                for j in range(2):
                    kb = 2 * pr + j
                    k0 = kb * P
                    scj = sc[:, j * QG:(j + 1) * QG]
                    nc.tensor.matmul(scj, xt0[:, k0:k0 + P], zt0[:, q0:q1],
                                     start=True, stop=False)
                    nc.tensor.matmul(scj, xt1[:, k0:k0 + P], zt1[:, q0:q1],
                                     start=False, stop=True)
                et = etp.tile([P, 2 * QG], BF16, tag="et")
                nc.scalar.activation(et[:, 0:QG], sc[:, 0:QG], AF.Exp)
                nc.scalar.activation(et[:, QG:2 * QG], sc[:, QG:2 * QG], AF.Exp)
                ets[pr] = et
                if qg == 0:
                    emit_v(2 * pr)
                    emit_v(2 * pr + 1)
                if pr == 2 and pending is not None:
                    emit_proj(*pending)
                    pending = None
                if pr >= 1:
                    emit_av(acc1, acc2, ets.pop(pr - 1), pr - 1)
            emit_av(acc1, acc2, ets.pop(npr - 1), npr - 1)

            ut0 = utp.tile([P, QG], BF16, tag="ut0")
            ut1 = utp.tile([DB, QG], BF16, tag="ut1")
            nc.vector.tensor_copy(ut0[:], acc1[:])
            if qg == nqg - 1:
                nc.scalar.copy(ut1[:], acc2[:])
            else:
                nc.vector.tensor_copy(ut1[:], acc2[:])
            pending = (qg, ut0, ut1)
        emit_proj(*pending)


_CACHE = {}


def _build(s_len=S):
    if s_len not in _CACHE:
        nc = bacc.Bacc("TRN2", target_bir_lowering=False, debug=False,
                       num_devices=NCORES)
        xt0_ext = nc.dram_tensor("xt0", [P, s_len], BF16, kind="ExternalInput")
        xt1_ext = nc.dram_tensor("xt1", [DB, s_len], BF16, kind="ExternalInput")
        w_exts = []
        for nm, rows in [("wz0", P), ("wz1", DB), ("wv0", P), ("wv1", DB),
                         ("wo0", P), ("wo1", DB)]:
            w_exts.append(nc.dram_tensor(nm, [rows, DA], BF16,
                                         kind="ExternalInput"))
        bo_ext = nc.dram_tensor("bo", [P, D], F32, kind="ExternalInput")
        out_ext = nc.dram_tensor("out", [s_len // 2, D], RS_DT,
                                 kind="ExternalOutput")
        rs_bufs = []
        for g in range(s_len // CROWS):
            ci = nc.dram_tensor(f"rs_in{g}", [CROWS, D], RS_DT)
            co = nc.dram_tensor(f"rs_out{g}", [CROWS // 2, D], RS_DT)
            rs_bufs.append((ci, co))
        exts = (xt0_ext, xt1_ext, *w_exts, bo_ext, out_ext, rs_bufs)
        with tile.TileContext(nc) as tc:
            with ExitStack() as ctx:
                _emit(ctx, tc, nc, exts, s_len)
        nc.compile()
        _CACHE[s_len] = nc
    return _CACHE[s_len]


def _prep_in_maps(m1, m2, Wq, bq, Wk, bk, Wv, bv, Wo, bo, s_len=S):
    bf = ml_dtypes.bfloat16
    sc = np.float64(1.0 / np.sqrt(D))
    wq_a = np.concatenate([Wq.T.astype(np.float64) * sc,
                           (bq.astype(np.float64) * sc)[None, :]], 0)
    wk_a = np.concatenate([Wk.T.astype(np.float64),
                           bk.astype(np.float64)[None, :]], 0)
    A = (wq_a @ wk_a.T).astype(bf)          # [201, 201]: scores = x̂ A x̂^T
    wv_p = np.zeros((DA, DA), np.float32)
    wv_p[:D, :D] = Wv.T
    wv_p[D, :D] = bv
    wv_p[D, D] = 1.0
    wv_p = wv_p.astype(bf)
    wo_p = np.zeros((DA, DA), np.float32)
    wo_p[:D, :D] = Wo.T
    wo_p[D, D] = 1.0
    wo_p = wo_p.astype(bf)
    bo_t = np.ascontiguousarray(
        np.broadcast_to((bo * 0.5).astype(np.float32), (P, D)))
    weights = {
        "wz0": np.ascontiguousarray(A[:P]),
        "wz1": np.ascontiguousarray(A[P:]),
        "wv0": np.ascontiguousarray(wv_p[:P]),
        "wv1": np.ascontiguousarray(wv_p[P:]),
        "wo0": np.ascontiguousarray(wo_p[:P]),
        "wo1": np.ascontiguousarray(wo_p[P:]),
        "bo": bo_t,
    }
    ones = np.ones((1, s_len), np.float32)
    in_maps = []
    for c in range(NCORES):
        b, m = c // 2, c % 2
        x = (m1 if m == 0 else m2)[b][:s_len]             # [s, 200] f32
        xt = np.concatenate([x.T, ones], 0).astype(bf)    # [201, s] bf16
        in_maps.append({
            "xt0": np.ascontiguousarray(xt[:P]),
            "xt1": np.ascontiguousarray(xt[P:]),
            **weights,
        })
    return in_maps


def _run(inputs, s_len=S, trace=False, tmpdir=None):
    m1 = np.asarray(inputs["modal1_input"], np.float32)
    m2 = np.asarray(inputs["modal2_input"], np.float32)
    args = [np.asarray(inputs[k], np.float32)
            for k in ("Wq", "bq", "Wk", "bk", "Wv", "bv", "Wo", "bo")]
    nc = _build(s_len)
    in_maps = _prep_in_maps(m1, m2, *args, s_len=s_len)
    kr = run_bass_kernel_spmd(nc, in_maps, core_ids=list(range(NCORES)),
                              trace=trace, tmpdir=tmpdir)
    res = kr.results
    out = np.empty((B, s_len, D), np.float32)
    csz = CROWS // 2
    nchunk = s_len // CROWS
    for b in range(B):
        e = np.asarray(res[2 * b]["out"], np.float32)
        o = np.asarray(res[2 * b + 1]["out"], np.float32)
        for g in range(nchunk):
            out[b, g * CROWS:g * CROWS + csz] = e[g * csz:(g + 1) * csz]
            out[b, g * CROWS + csz:(g + 1) * CROWS] = o[g * csz:(g + 1) * csz]
    return out, kr


def kernel(**inputs):
    out, _ = _run(inputs)
    return out
